# revision 10
# baseline (speedup 1.0000x reference)
"""Trainium2 Bass kernel for nn_AttentionSheafLearner.

Computation:  maps = x[row] @ W[:, :C].T + x[col] @ W[:, C:].T    [E, 25]
              out  = eye(5) - softmax(maps.reshape(E, 5, 5), axis=-1)

Strategy (8 NeuronCores, SPMD):
  - Precompute z[n] = [x[n] @ Wr.T | x[n] @ Wc.T | 0-pad]  (64 f32 = 256B rows)
    on device with PE matmuls; store per node-half tables in DRAM.
  - Edges are sharded by VALUE class: nodes split in two halves (A = <25088),
    edge class = (row_half, col_half); each of the 4 classes is handled by 2
    cores. This keeps per-core gather indices < 25088 so they fit int16, the
    index dtype of the SWDGE dma_gather instruction.
  - Per 6528-edge chunk: dma_gather 256B z rows for row and col endpoints,
    DVE add -> maps, ScalarE exp, DVE strided reduce / reciprocal /
    broadcast-mul, out = eye - sm, store.
  - Host re-permutes the output rows back to original edge order.
"""

import os

import numpy as np

# problem sizes (hardcoded per contract)
N = 50000
C = 128
D = 5
DD = D * D          # 25
E = 1_600_000
NCORES = 8
P = 128

HALF = 25088        # nodes per half (padded; 2*HALF >= N)
ZW = 64             # z row width in f32 (256B, dma_gather elem size)
NCH_H = HALF // P   # 196 node chunks per half

T = 1596            # edge cols per partition per core => capacity 204288/core
EPC = P * T         # 204288
CHKC = 57           # edge cols per partition per chunk
CHK = P * CHKC      # 7296 edges per chunk
NCHUNK = T // CHKC  # 28
IDXW = CHK // 16    # 456 idx cols per chunk
IDX_COLS = NCHUNK * IDXW  # 12768

_XBLK = 28          # node chunks per xT DMA block
_ZGRP = 14          # node chunks per z store group


def _build_nc():
    from contextlib import ExitStack

    import concourse.bacc as bacc
    import concourse.mybir as mybir
    import concourse.tile as tile

    f32 = mybir.dt.float32
    bf16 = mybir.dt.bfloat16
    i16 = mybir.dt.int16

    nc = bacc.Bacc(
        "TRN2",
        target_bir_lowering=False,
        debug=False,
        enable_asserts=False,
        num_devices=NCORES,
        num_swdge_queues=4,
    )

    xt_r_d = nc.dram_tensor("xt_r", [P, HALF], f32, kind="ExternalInput")
    xt_c_d = nc.dram_tensor("xt_c", [P, HALF], f32, kind="ExternalInput")
    w_d = nc.dram_tensor("w", [P, 2 * DD], f32, kind="ExternalInput")
    ridx_d = nc.dram_tensor("ridx", [P, IDX_COLS], i16, kind="ExternalInput")
    cidx_d = nc.dram_tensor("cidx", [P, IDX_COLS], i16, kind="ExternalInput")
    eye_d = nc.dram_tensor("eye", [P, DD], f32, kind="ExternalInput")
    z_r_d = nc.dram_tensor("z_r", [HALF, ZW], f32)
    z_c_d = nc.dram_tensor("z_c", [HALF, ZW], f32)
    out_d = nc.dram_tensor("out", [EPC, DD], f32, kind="ExternalOutput")

    oview = out_d.ap().rearrange("(p t) d -> p (t d)", p=P)  # [128, T*25]

    with tile.TileContext(nc) as tc, ExitStack() as ctx:
        const_pool = ctx.enter_context(tc.tile_pool(name="const", bufs=1))
        w_tile = const_pool.tile([P, 2 * DD], f32)
        nc.sync.dma_start(w_tile[:], w_d.ap())
        eye_tile = const_pool.tile([P, DD], f32)
        nc.sync.dma_start(eye_tile[:], eye_d.ap())

        # ---- stage A: z tables ----
        with ExitStack() as actx:
            xt_pool = actx.enter_context(tc.tile_pool(name="xt", bufs=2))
            z_pool = actx.enter_context(tc.tile_pool(name="zsb", bufs=3))
            ps_pool = actx.enter_context(
                tc.tile_pool(name="ps", bufs=4, space="PSUM")
            )
            for xt_d, z_d in ((xt_r_d, z_r_d), (xt_c_d, z_c_d)):
                zview = z_d.ap().rearrange("(i p) d -> i p d", p=P)
                for blk in range(NCH_H // _XBLK):  # 7
                    xt_tile = xt_pool.tile([P, _XBLK * P], f32)
                    nc.sync.dma_start(
                        xt_tile[:],
                        xt_d.ap()[:, blk * _XBLK * P:(blk + 1) * _XBLK * P],
                    )
                    for grp in range(_XBLK // _ZGRP):  # 2
                        z_sb = z_pool.tile([P, _ZGRP * ZW], f32)
                        nc.gpsimd.memset(
                            z_sb[:].rearrange("p (i d) -> p i d", i=_ZGRP)[
                                :, :, 2 * DD:
                            ],
                            0.0,
                        )
                        for j in range(_ZGRP):
                            jj = grp * _ZGRP + j
                            ps = ps_pool.tile([P, 2 * DD], f32, space="PSUM")
                            nc.tensor.matmul(
                                ps[:],
                                xt_tile[:, jj * P:(jj + 1) * P],
                                w_tile[:],
                                start=True,
                                stop=True,
                            )
                            nc.vector.tensor_copy(
                                z_sb[:, j * ZW:j * ZW + 2 * DD], ps[:]
                            )
                        i0 = blk * _XBLK + grp * _ZGRP
                        nc.sync.dma_start(
                            zview[i0:i0 + _ZGRP, :, :].rearrange("i p d -> p i d"),
                            z_sb[:].rearrange("p (i d) -> p i d", i=_ZGRP),
                        )

        # ---- stage B: gather + softmax ----
        g_pool = ctx.enter_context(tc.tile_pool(name="g", bufs=4))
        i_pool = ctx.enter_context(tc.tile_pool(name="ix", bufs=4))
        m_pool = ctx.enter_context(tc.tile_pool(name="m", bufs=3))
        e_pool = ctx.enter_context(tc.tile_pool(name="e", bufs=3))
        s_pool = ctx.enter_context(tc.tile_pool(name="s", bufs=3))
        o_pool = ctx.enter_context(tc.tile_pool(name="o", bufs=3))
        for ch in range(NCHUNK):
            ri = i_pool.tile([P, IDXW], i16, tag="ri")
            nc.sync.dma_start(ri[:], ridx_d.ap()[:, ch * IDXW:(ch + 1) * IDXW])
            ci = i_pool.tile([P, IDXW], i16, tag="ci")
            nc.sync.dma_start(ci[:], cidx_d.ap()[:, ch * IDXW:(ch + 1) * IDXW])
            g_r = g_pool.tile([P, CHKC * ZW], f32, tag="gr")
            nc.gpsimd.dma_gather(
                out_ap=g_r[:].rearrange("p (u d) -> p u d", d=ZW),
                in_ap=z_r_d.ap(),
                idxs_ap=ri[:],
                num_idxs=CHK,
                num_idxs_reg=CHK,
                elem_size=ZW,
                single_packet=False,
                queue_num=(2 * ch) % 4,
            )
            g_c = g_pool.tile([P, CHKC * ZW], f32, tag="gc")
            nc.gpsimd.dma_gather(
                out_ap=g_c[:].rearrange("p (u d) -> p u d", d=ZW),
                in_ap=z_c_d.ap(),
                idxs_ap=ci[:],
                num_idxs=CHK,
                num_idxs_reg=CHK,
                elem_size=ZW,
                single_packet=False,
                queue_num=(2 * ch + 1) % 4,
            )
            m = m_pool.tile([P, CHKC * DD], f32)
            nc.vector.tensor_tensor(
                out=m[:].rearrange("p (u d) -> p u d", d=DD),
                in0=g_r[:].rearrange("p (u d) -> p u d", d=ZW)[:, :, 0:DD],
                in1=g_c[:].rearrange("p (u d) -> p u d", d=ZW)[:, :, DD:2 * DD],
                op=mybir.AluOpType.add,
            )
            et = e_pool.tile([P, CHKC * DD], f32)
            nc.scalar.activation(et[:], m[:], mybir.ActivationFunctionType.Exp)
            e3 = et[:].rearrange("p (t d) -> p t d", d=D)  # [128, CHKC*5, 5]
            s = s_pool.tile([P, CHKC * D], f32, tag="s")
            nc.vector.reduce_sum(s[:], e3, axis=mybir.AxisListType.X)
            r = s_pool.tile([P, CHKC * D], f32, tag="r")
            nc.vector.reciprocal(r[:], s[:])
            o = o_pool.tile([P, CHKC * DD], f32)
            nc.vector.tensor_tensor(
                out=o[:].rearrange("p (t d) -> p t d", d=D),
                in0=e3,
                in1=r[:].unsqueeze(2).to_broadcast([P, CHKC * D, D]),
                op=mybir.AluOpType.mult,
            )
            nc.vector.tensor_tensor(
                out=o[:].rearrange("p (t d) -> p t d", d=DD),
                in0=eye_tile[:].unsqueeze(1).to_broadcast([P, CHKC, DD]),
                in1=o[:].rearrange("p (t d) -> p t d", d=DD),
                op=mybir.AluOpType.subtract,
            )
            nc.sync.dma_start(oview[:, ch * CHKC * DD:(ch + 1) * CHKC * DD], o[:])

    nc.compile()
    return nc


def _host_prep(x, W, edge_index):
    """Shard edges by (row_half, col_half) class across cores; build per-core
    inputs. Returns (in_maps, slot_maps, spill) where slot_maps[c] holds the
    original edge id for each real (non-pad) list position, and spill is a
    list of original edge ids handled on host."""
    x = np.asarray(x, dtype=np.float32)
    W = np.asarray(W, dtype=np.float32)
    ei = np.asarray(edge_index)
    row = ei[0].astype(np.int64)
    col = ei[1].astype(np.int64)

    xt = np.zeros((P, 2 * HALF), dtype=np.float32)
    xt[:, :N] = x.T
    xt_half = [np.ascontiguousarray(xt[:, :HALF]), np.ascontiguousarray(xt[:, HALF:])]

    w = np.zeros((P, 2 * DD), dtype=np.float32)
    w[:, :DD] = W[:, :C].T
    w[:, DD:2 * DD] = W[:, C:].T
    eye = np.ascontiguousarray(
        np.broadcast_to(np.eye(D, dtype=np.float32).reshape(1, DD), (P, DD))
    )

    cls = (row >= HALF).astype(np.int64) * 2 + (col >= HALF)
    order = np.argsort(cls, kind="stable")
    counts = np.bincount(cls, minlength=4)
    starts = np.concatenate([[0], np.cumsum(counts)])

    in_maps = []
    slot_maps = []
    spill = []
    for core in range(NCORES):
        k = core // 2
        half_r, half_c = k >> 1, k & 1
        cls_edges = order[starts[k]:starts[k + 1]]
        sub = cls_edges[core % 2::2]          # interleave class across 2 cores
        if len(sub) > EPC:
            spill.extend(sub[EPC:].tolist())
            sub = sub[:EPC]
        m = len(sub)
        lr = np.zeros(EPC, dtype=np.int16)
        lc = np.zeros(EPC, dtype=np.int16)
        lr[:m] = (row[sub] - half_r * HALF).astype(np.int16)
        lc[:m] = (col[sub] - half_c * HALF).astype(np.int16)

        def wrap(loc):
            # list position l = ch*CHK + p*CHKC + u  ->  gather pos i = u*128+p
            a = loc.reshape(NCHUNK, P, CHKC).transpose(0, 2, 1)  # [ch, u, p]
            a = a.reshape(NCHUNK, CHK)                           # gather order
            a = a.reshape(NCHUNK, IDXW, 16)
            a = a.transpose(2, 0, 1).reshape(16, IDX_COLS)       # [16, cols]
            return np.ascontiguousarray(np.tile(a, (8, 1)))

        in_maps.append(
            {
                "xt_r": xt_half[half_r],
                "xt_c": xt_half[half_c],
                "w": w,
                "eye": eye,
                "ridx": wrap(lr),
                "cidx": wrap(lc),
            }
        )
        slot_maps.append(sub)
    return in_maps, slot_maps, spill


def _host_spill_compute(x, W, edge_index, ids):
    row = np.asarray(edge_index[0])[ids].astype(np.int64)
    col = np.asarray(edge_index[1])[ids].astype(np.int64)
    x = np.asarray(x, dtype=np.float32)
    W = np.asarray(W, dtype=np.float32)
    maps = (x[row] @ W[:, :C].T + x[col] @ W[:, C:].T).reshape(-1, D, D)
    em = np.exp(maps - maps.max(-1, keepdims=True))
    sm = em / em.sum(-1, keepdims=True)
    return np.eye(D, dtype=np.float32)[None] - sm


LAST_EXEC_NS = None


def kernel(x, W, edge_index):
    global LAST_EXEC_NS
    from concourse.bass_utils import run_bass_kernel_spmd

    nc = _build_nc()
    in_maps, slot_maps, spill = _host_prep(x, W, edge_index)
    trace = os.environ.get("KERNEL_TRACE", "0") == "1"
    br = run_bass_kernel_spmd(
        nc,
        in_maps,
        core_ids=list(range(NCORES)),
        trace=trace,
    )
    LAST_EXEC_NS = br.exec_time_ns

    out = np.empty((E, DD), dtype=np.float32)
    for core in range(NCORES):
        res = br.results[core]["out"]                  # [EPC, 25], p-major slots
        ordered = (
            res.reshape(P, NCHUNK, CHKC, DD)
            .transpose(1, 0, 2, 3)
            .reshape(EPC, DD)
        )                                              # list-position order
        ids = slot_maps[core]
        out[ids] = ordered[: len(ids)]
    if spill:
        out[np.asarray(spill)] = _host_spill_compute(
            x, W, edge_index, np.asarray(spill)
        ).reshape(-1, DD)
    return out.reshape(E, D, D).astype(np.float32)


# revision 11
# speedup vs baseline: 1.0032x; 1.0032x over previous
"""Trainium2 Bass kernel for nn_AttentionSheafLearner.

Computation:  maps = x[row] @ W[:, :C].T + x[col] @ W[:, C:].T    [E, 25]
              out  = eye(5) - softmax(maps.reshape(E, 5, 5), axis=-1)

Strategy (8 NeuronCores, SPMD):
  - Precompute z[n] = [x[n] @ Wr.T | x[n] @ Wc.T | 0-pad]  (64 f32 = 256B rows)
    on device with PE matmuls; store per node-half tables in DRAM.
  - Edges are sharded by VALUE class: nodes split in two halves (A = <25088),
    edge class = (row_half, col_half); each of the 4 classes is handled by 2
    cores. This keeps per-core gather indices < 25088 so they fit int16, the
    index dtype of the SWDGE dma_gather instruction.
  - Per 6528-edge chunk: dma_gather 256B z rows for row and col endpoints,
    DVE add -> maps, ScalarE exp, DVE strided reduce / reciprocal /
    broadcast-mul, out = eye - sm, store.
  - Host re-permutes the output rows back to original edge order.
"""

import os

import numpy as np

# problem sizes (hardcoded per contract)
N = 50000
C = 128
D = 5
DD = D * D          # 25
E = 1_600_000
NCORES = 8
P = 128

HALF = 25088        # nodes per half (padded; 2*HALF >= N)
ZW = 64             # z row width in f32 (256B, dma_gather elem size)
NCH_H = HALF // P   # 196 node chunks per half

T = 1596            # edge cols per partition per core => capacity 204288/core
EPC = P * T         # 204288
CHKC = 57           # edge cols per partition per chunk
CHK = P * CHKC      # 7296 edges per chunk
NCHUNK = T // CHKC  # 28
IDXW = CHK // 16    # 456 idx cols per chunk
IDX_COLS = NCHUNK * IDXW  # 12768

_XBLK = 28          # node chunks per xT DMA block
_ZGRP = 14          # node chunks per z store group


def _build_nc():
    from contextlib import ExitStack

    import concourse.bacc as bacc
    import concourse.mybir as mybir
    import concourse.tile as tile

    f32 = mybir.dt.float32
    bf16 = mybir.dt.bfloat16
    i16 = mybir.dt.int16

    nc = bacc.Bacc(
        "TRN2",
        target_bir_lowering=False,
        debug=False,
        enable_asserts=False,
        num_devices=NCORES,
        num_swdge_queues=4,
    )

    xt_r_d = nc.dram_tensor("xt_r", [P, HALF], f32, kind="ExternalInput")
    xt_c_d = nc.dram_tensor("xt_c", [P, HALF], f32, kind="ExternalInput")
    w_d = nc.dram_tensor("w", [P, 2 * DD], f32, kind="ExternalInput")
    ridx_d = nc.dram_tensor("ridx", [P, IDX_COLS], i16, kind="ExternalInput")
    cidx_d = nc.dram_tensor("cidx", [P, IDX_COLS], i16, kind="ExternalInput")
    eye_d = nc.dram_tensor("eye", [P, DD], f32, kind="ExternalInput")
    z_r_d = nc.dram_tensor("z_r", [HALF, ZW], f32)
    z_c_d = nc.dram_tensor("z_c", [HALF, ZW], f32)
    out_d = nc.dram_tensor("out", [EPC, DD], f32, kind="ExternalOutput")

    oview = out_d.ap().rearrange("(p t) d -> p (t d)", p=P)  # [128, T*25]

    with tile.TileContext(nc) as tc, ExitStack() as ctx:
        const_pool = ctx.enter_context(tc.tile_pool(name="const", bufs=1))
        w_tile = const_pool.tile([P, 2 * DD], f32)
        nc.sync.dma_start(w_tile[:], w_d.ap())
        eye_tile = const_pool.tile([P, DD], f32)
        nc.sync.dma_start(eye_tile[:], eye_d.ap())

        # ---- stage A: z tables ----
        with ExitStack() as actx:
            xt_pool = actx.enter_context(tc.tile_pool(name="xt", bufs=2))
            z_pool = actx.enter_context(tc.tile_pool(name="zsb", bufs=3))
            ps_pool = actx.enter_context(
                tc.tile_pool(name="ps", bufs=4, space="PSUM")
            )
            for xt_d, z_d in ((xt_r_d, z_r_d), (xt_c_d, z_c_d)):
                zview = z_d.ap().rearrange("(i p) d -> i p d", p=P)
                for blk in range(NCH_H // _XBLK):  # 7
                    xt_tile = xt_pool.tile([P, _XBLK * P], f32)
                    nc.sync.dma_start(
                        xt_tile[:],
                        xt_d.ap()[:, blk * _XBLK * P:(blk + 1) * _XBLK * P],
                    )
                    for grp in range(_XBLK // _ZGRP):  # 2
                        z_sb = z_pool.tile([P, _ZGRP * ZW], f32)
                        nc.gpsimd.memset(
                            z_sb[:].rearrange("p (i d) -> p i d", i=_ZGRP)[
                                :, :, 2 * DD:
                            ],
                            0.0,
                        )
                        for j in range(_ZGRP):
                            jj = grp * _ZGRP + j
                            ps = ps_pool.tile([P, 2 * DD], f32, space="PSUM")
                            nc.tensor.matmul(
                                ps[:],
                                xt_tile[:, jj * P:(jj + 1) * P],
                                w_tile[:],
                                start=True,
                                stop=True,
                            )
                            nc.vector.tensor_copy(
                                z_sb[:, j * ZW:j * ZW + 2 * DD], ps[:]
                            )
                        i0 = blk * _XBLK + grp * _ZGRP
                        nc.sync.dma_start(
                            zview[i0:i0 + _ZGRP, :, :].rearrange("i p d -> p i d"),
                            z_sb[:].rearrange("p (i d) -> p i d", i=_ZGRP),
                        )

        # ---- stage B: gather + softmax ----
        g_pool = ctx.enter_context(tc.tile_pool(name="g", bufs=4))
        i_pool = ctx.enter_context(tc.tile_pool(name="ix", bufs=4))
        m_pool = ctx.enter_context(tc.tile_pool(name="m", bufs=2))
        e_pool = ctx.enter_context(tc.tile_pool(name="e", bufs=2))
        s_pool = ctx.enter_context(tc.tile_pool(name="s", bufs=2))
        o_pool = ctx.enter_context(tc.tile_pool(name="o", bufs=2))
        for ch in range(NCHUNK):
            ri = i_pool.tile([P, IDXW], i16, tag="ri")
            nc.sync.dma_start(ri[:], ridx_d.ap()[:, ch * IDXW:(ch + 1) * IDXW])
            ci = i_pool.tile([P, IDXW], i16, tag="ci")
            nc.sync.dma_start(ci[:], cidx_d.ap()[:, ch * IDXW:(ch + 1) * IDXW])
            g_r = g_pool.tile([P, CHKC * ZW], f32, tag="gr")
            nc.gpsimd.dma_gather(
                out_ap=g_r[:].rearrange("p (u d) -> p u d", d=ZW),
                in_ap=z_r_d.ap(),
                idxs_ap=ri[:],
                num_idxs=CHK,
                num_idxs_reg=CHK,
                elem_size=ZW,
                single_packet=False,
                queue_num=(2 * ch) % 4,
            )
            g_c = g_pool.tile([P, CHKC * ZW], f32, tag="gc")
            nc.gpsimd.dma_gather(
                out_ap=g_c[:].rearrange("p (u d) -> p u d", d=ZW),
                in_ap=z_c_d.ap(),
                idxs_ap=ci[:],
                num_idxs=CHK,
                num_idxs_reg=CHK,
                elem_size=ZW,
                single_packet=False,
                queue_num=(2 * ch + 1) % 4,
            )
            m = m_pool.tile([P, CHKC * DD], f32)
            nc.vector.tensor_tensor(
                out=m[:].rearrange("p (u d) -> p u d", d=DD),
                in0=g_r[:].rearrange("p (u d) -> p u d", d=ZW)[:, :, 0:DD],
                in1=g_c[:].rearrange("p (u d) -> p u d", d=ZW)[:, :, DD:2 * DD],
                op=mybir.AluOpType.add,
            )
            et = e_pool.tile([P, CHKC * DD], f32)
            nc.scalar.activation(et[:], m[:], mybir.ActivationFunctionType.Exp)
            e3 = et[:].rearrange("p (t d) -> p t d", d=D)  # [128, CHKC*5, 5]
            s = s_pool.tile([P, CHKC * D], f32, tag="s")
            nc.vector.reduce_sum(s[:], e3, axis=mybir.AxisListType.X)
            r = s_pool.tile([P, CHKC * D], f32, tag="r")
            nc.vector.reciprocal(r[:], s[:])
            o = o_pool.tile([P, CHKC * DD], f32)
            nc.vector.tensor_tensor(
                out=o[:].rearrange("p (t d) -> p t d", d=D),
                in0=e3,
                in1=r[:].unsqueeze(2).to_broadcast([P, CHKC * D, D]),
                op=mybir.AluOpType.mult,
            )
            nc.vector.tensor_tensor(
                out=o[:].rearrange("p (t d) -> p t d", d=DD),
                in0=eye_tile[:].unsqueeze(1).to_broadcast([P, CHKC, DD]),
                in1=o[:].rearrange("p (t d) -> p t d", d=DD),
                op=mybir.AluOpType.subtract,
            )
            nc.sync.dma_start(oview[:, ch * CHKC * DD:(ch + 1) * CHKC * DD], o[:])

    nc.compile()
    return nc


def _host_prep(x, W, edge_index):
    """Shard edges by (row_half, col_half) class across cores; build per-core
    inputs. Returns (in_maps, slot_maps, spill) where slot_maps[c] holds the
    original edge id for each real (non-pad) list position, and spill is a
    list of original edge ids handled on host."""
    x = np.asarray(x, dtype=np.float32)
    W = np.asarray(W, dtype=np.float32)
    ei = np.asarray(edge_index)
    row = ei[0].astype(np.int64)
    col = ei[1].astype(np.int64)

    xt = np.zeros((P, 2 * HALF), dtype=np.float32)
    xt[:, :N] = x.T
    xt_half = [np.ascontiguousarray(xt[:, :HALF]), np.ascontiguousarray(xt[:, HALF:])]

    w = np.zeros((P, 2 * DD), dtype=np.float32)
    w[:, :DD] = W[:, :C].T
    w[:, DD:2 * DD] = W[:, C:].T
    eye = np.ascontiguousarray(
        np.broadcast_to(np.eye(D, dtype=np.float32).reshape(1, DD), (P, DD))
    )

    cls = (row >= HALF).astype(np.int64) * 2 + (col >= HALF)
    order = np.argsort(cls, kind="stable")
    counts = np.bincount(cls, minlength=4)
    starts = np.concatenate([[0], np.cumsum(counts)])

    in_maps = []
    slot_maps = []
    spill = []
    for core in range(NCORES):
        k = core // 2
        half_r, half_c = k >> 1, k & 1
        cls_edges = order[starts[k]:starts[k + 1]]
        sub = cls_edges[core % 2::2]          # interleave class across 2 cores
        if len(sub) > EPC:
            spill.extend(sub[EPC:].tolist())
            sub = sub[:EPC]
        m = len(sub)
        lr = np.zeros(EPC, dtype=np.int16)
        lc = np.zeros(EPC, dtype=np.int16)
        lr[:m] = (row[sub] - half_r * HALF).astype(np.int16)
        lc[:m] = (col[sub] - half_c * HALF).astype(np.int16)

        def wrap(loc):
            # list position l = ch*CHK + p*CHKC + u  ->  gather pos i = u*128+p
            a = loc.reshape(NCHUNK, P, CHKC).transpose(0, 2, 1)  # [ch, u, p]
            a = a.reshape(NCHUNK, CHK)                           # gather order
            a = a.reshape(NCHUNK, IDXW, 16)
            a = a.transpose(2, 0, 1).reshape(16, IDX_COLS)       # [16, cols]
            return np.ascontiguousarray(np.tile(a, (8, 1)))

        in_maps.append(
            {
                "xt_r": xt_half[half_r],
                "xt_c": xt_half[half_c],
                "w": w,
                "eye": eye,
                "ridx": wrap(lr),
                "cidx": wrap(lc),
            }
        )
        slot_maps.append(sub)
    return in_maps, slot_maps, spill


def _host_spill_compute(x, W, edge_index, ids):
    row = np.asarray(edge_index[0])[ids].astype(np.int64)
    col = np.asarray(edge_index[1])[ids].astype(np.int64)
    x = np.asarray(x, dtype=np.float32)
    W = np.asarray(W, dtype=np.float32)
    maps = (x[row] @ W[:, :C].T + x[col] @ W[:, C:].T).reshape(-1, D, D)
    em = np.exp(maps - maps.max(-1, keepdims=True))
    sm = em / em.sum(-1, keepdims=True)
    return np.eye(D, dtype=np.float32)[None] - sm


LAST_EXEC_NS = None


def kernel(x, W, edge_index):
    global LAST_EXEC_NS
    from concourse.bass_utils import run_bass_kernel_spmd

    nc = _build_nc()
    in_maps, slot_maps, spill = _host_prep(x, W, edge_index)
    trace = os.environ.get("KERNEL_TRACE", "0") == "1"
    br = run_bass_kernel_spmd(
        nc,
        in_maps,
        core_ids=list(range(NCORES)),
        trace=trace,
    )
    LAST_EXEC_NS = br.exec_time_ns

    out = np.empty((E, DD), dtype=np.float32)
    for core in range(NCORES):
        res = br.results[core]["out"]                  # [EPC, 25], p-major slots
        ordered = (
            res.reshape(P, NCHUNK, CHKC, DD)
            .transpose(1, 0, 2, 3)
            .reshape(EPC, DD)
        )                                              # list-position order
        ids = slot_maps[core]
        out[ids] = ordered[: len(ids)]
    if spill:
        out[np.asarray(spill)] = _host_spill_compute(
            x, W, edge_index, np.asarray(spill)
        ).reshape(-1, DD)
    return out.reshape(E, D, D).astype(np.float32)
